# revision 2
# baseline (speedup 1.0000x reference)
"""Trainium2 Bass kernel for nn_DilatedSpatioTemporalGCN — v3.

Same math as v2 (adjacency collapses to (J+I)/513; per-layer
Z = c2*(X + 1 colsum(X)^T) @ M_l + bias_row, g = sigmoid(Z),
r = relu(dilated causal conv(g) + bc), X += r; tiny 3-way attention tail),
with a restructured schedule:

- emb tiles stay bf16 and feed matmuls directly (no upconverts); Z weights
  are duplicated: bf16 copies multiply emb, f32r copies multiply the f32r
  residual r tiles. Host precomputes the emb colsum columns.
- conv taps are merged by chunk-carry: all (k, q_in->q_out) cells for a
  given carry land in one 128x128 lhsT, so a chunk's conv is 2 matmuls.
- layer 2 is fully packed: chunk c lives on partitions 32c:32c+32 of one
  [96,512] PSUM tile via block-diagonal / zero-padded lhsT weights (dst
  partition base stays 0), one sigmoid, one K-stacked conv matmul.
- inputs: one HWDGE DMA carries the weight head + emb c0; emb c1/c2 and
  the packed l2 rows arrive via gpsimd dma_gather (parallel SWDGE channel);
  remaining weights stream on later HWDGE slots in need order.

Sharding: data-parallel over batch (4 elements) on cores 0-3; cores 4-7
run duplicate batches (harmless, keeps all 8 cores uniform).
"""

import os
import numpy as np
import ml_dtypes
from contextlib import ExitStack

import concourse.bacc as bacc
import concourse.tile as tile
from concourse import mybir
from concourse.bass_utils import run_bass_kernel_spmd

F32 = mybir.dt.float32
F32R = mybir.dt.float32r
BF16 = mybir.dt.bfloat16
I16 = mybir.dt.int16
AF = mybir.ActivationFunctionType
ALU = mybir.AluOpType
AX = mybir.AxisListType

BSZ, T, FD, N = 4, 12, 32, 512
L, K = 3, 3
DIL = (1, 2, 4)
NTHI = 3
NCHUNK = 4

# All weight containers are plain F32R dram tensors (host f32 values; the
# runtime rounds to f32r's ~12-bit mantissa; memory layout is IEEE f32
# bytes). bf16 operands for the emb-side matmuls are STRIDE-2 bitcast views
# picking the high u16 of each f32r value (== bf16 truncation). Offsets are
# f32r columns.
ZR0 = 0             # 128: blockdiag Mc0 f32r (zwb0 = bf16 view of it)
BRT = 128           # 128: [3 rows, 128] f32r tiled bias rows per layer
ECS = 256           # 6: emb colsums per chunk ([128,2] f32r, col1 zero)
IDN = 262           # 32: [32,32] f32r identity
WFC = 294           # 4: [128,4] f32 bitcast: bc0..2 tiled; ba (rows 0:32)
OH = 298            # 4: [4,4] f32r one-hot cols
VV = 302            # 2: [32,2] f32r v (col1 zero)
ECS2 = 304          # 2: [96,2] f32r packed l2 colsums (col1 zero)
WH_COLS = 306

# wm (slot2): CV0A 0:128, CV0B 128:256, ZR1 256:384 (blockdiag Mc1)
WM_COLS = 384
# wm2 (slot3): CV1A 0:128, CV1B 128:256, ZWR2BD 256:352 ([96,96] blockdiag
#   Mc2; zwb2bd = bf16 view), WA 352:384
WM2_COLS = 384
# wm3 (slot4): ZWR2PAD 0:288 ([32,96] x3), CV2 288:320 ([96,32]),
#   BRT2 320:416 ([1,96])
WM3_COLS = 416

_CACHE = {}


def _conv_cells(l):
    """Cells (q_in, q_out, k, carry) for layer l in the reversed-group
    layout; carry in {0,-1}."""
    d = DIL[l]
    cells = []
    for k in range(K):
        delta = (K - 1 - k) * d
        for q_out in range(4):
            a = 3 - q_out
            ap_ = (a - delta) % 4
            carry = (a - delta - ap_) // 4
            cells.append((3 - ap_, q_out, k, carry))
    return cells


def _prune_redundant_act_loads(nc):
    for blk in nc.main_func.blocks:
        insts = blk.instructions
        dead = []
        prev_load = None
        for inst in insts:
            if isinstance(inst, mybir.InstLoadActFuncSet):
                if prev_load is not None:
                    dead.append(prev_load)
                prev_load = inst
            elif isinstance(inst, mybir.InstActivation):
                prev_load = None
        for inst in dead:
            si = getattr(inst, "sync_info", None)
            if si is not None and (len(si.on_wait) > 0 or len(si.on_update) > 0):
                continue
            insts.remove(inst)


def _build_nc():
    nc = bacc.Bacc("TRN2", target_bir_lowering=False, debug=False)
    wh_d = nc.dram_tensor("wh", [128, WH_COLS], F32R, kind="ExternalInput").ap()
    eg_d = nc.dram_tensor("eg", [352, 512], BF16, kind="ExternalInput").ap()
    e0_d = nc.dram_tensor("e0", [128, 512], BF16, kind="ExternalInput").ap()
    wm_d = nc.dram_tensor("wm", [128, WM_COLS], F32R, kind="ExternalInput").ap()
    wm2_d = nc.dram_tensor("wm2", [128, WM2_COLS], F32R, kind="ExternalInput").ap()
    wm3_d = nc.dram_tensor("wm3", [128, WM3_COLS], F32R, kind="ExternalInput").ap()
    y_d = nc.dram_tensor("y", [N, FD], F32, kind="ExternalOutput").ap()
    debug = bool(int(os.environ.get("K_DEBUG", "0")))
    if debug:
        dbg = {}
        for nm, shape in [("emb0", [128, 512]), ("embs1", [128, 512]),
                          ("embl2", [128, 512]), ("g00", [128, 512]),
                          ("r00", [128, 514]), ("bv0", [128, 3]),
                          ("g2", [96, 512]), ("rs2", [32, 512]),
                          ("g10", [128, 512]), ("r10", [128, 514]),
                          ("r12", [128, 514]), ("bv2d", [96, 1]),
                          ("scat", [32, 1536]), ("esb", [128, 24]),
                          ("prt", [128, 384]), ("idxs", [128, 32]),
                          ("zwb0", [128, 128]), ("cv0a", [128, 128])]:
            dt = I16 if nm == "idxs" else F32
            dbg[nm] = nc.dram_tensor("dbg_" + nm, shape, dt,
                                     kind="ExternalOutput").ap()

    with tile.TileContext(nc) as tc, ExitStack() as ctx:
        ctx.enter_context(nc.allow_low_precision(
            "bf16 emb path validated to 3e-4 scale error; f32r elsewhere"))
        wpool = ctx.enter_context(tc.tile_pool(name="w", bufs=1))
        spool = ctx.enter_context(tc.tile_pool(name="s", bufs=1))
        ppool_z = ctx.enter_context(tc.tile_pool(name="pz", bufs=3, space="PSUM"))
        ppool_c = ctx.enter_context(tc.tile_pool(name="pc", bufs=3, space="PSUM"))
        ppool_s = ctx.enter_context(tc.tile_pool(name="psm", bufs=2, space="PSUM"))
        ppool_k = ctx.enter_context(tc.tile_pool(name="pk", bufs=1, space="PSUM"))

        # ---------------- persistent tiles ----------------
        wh = wpool.tile([128, WH_COLS], F32R)
        wm = wpool.tile([128, WM_COLS], F32R)
        wm2 = wpool.tile([128, WM2_COLS], F32R)
        wm3 = wpool.tile([128, WM3_COLS], F32R)
        embs0 = wpool.tile([128, 512], BF16, name="embs0")
        embs1 = wpool.tile([128, 512], BF16, name="embs1")
        embs2 = wpool.tile([128, 512], BF16, name="embs2")
        embl2 = wpool.tile([128, 512], BF16, name="embl2")
        idxs = spool.tile([128, 24], I16, name="idxs")
        pcol = spool.tile([128, 2], I16, name="pcol")
        pcolf = spool.tile([128, 1], F32, name="pcolf")
        zeros = spool.tile([128, N], F32, name="zeros")

        gps = [[wpool.tile([128, N], F32R, name=f"g{l}{c}") for c in range(NTHI)]
               for l in range(2)]
        g2all = wpool.tile([96, N], F32R, name="g2all")
        rs = [[wpool.tile([128, N + 2], F32R, name=f"r{l}{c}") for c in range(NTHI)]
              for l in range(2)]
        rs2 = wpool.tile([32, N], F32R, name="rs2")
        bvs = [spool.tile([128, NTHI], F32, name=f"bv{l}") for l in range(2)]
        bv2 = spool.tile([96, 1], F32, name="bv2")
        s_cat = spool.tile([32, L * N], F32R, name="s_cat")
        e_sb = spool.tile([128, NCHUNK, 6], F32)
        es = spool.tile([128, NCHUNK], F32)
        ri = spool.tile([128, NCHUNK], F32)
        y_sb = spool.tile([128, NCHUNK, FD], F32)
        tmps = [spool.tile([128, FD], F32, name=f"ytmp{i}") for i in range(3)]
        u_sb = spool.tile([128, NCHUNK, 6], F32, name="u_sb")
        un = spool.tile([128, NCHUNK, 6], F32, name="un")
        prt_sb = spool.tile([128, NCHUNK, L, FD], F32, name="prt_sb")
        pkt = ppool_k.tile([128, NCHUNK * L * FD + NCHUNK * 6], F32,
                           tag="prt", name="pkt")
        prt = pkt[:, 0:NCHUNK * L * FD].rearrange(
            "p (c l f) -> p c l f", c=NCHUNK, l=L)
        psvT = pkt[:, NCHUNK * L * FD:].rearrange(
            "p (c s) -> p c s", c=NCHUNK)

        # weight slices; bf16 operands are stride-2 (high-u16) views
        def bfview(region):
            return region.bitcast(BF16).rearrange(
                "p (c two) -> p two c", two=2)[:, 1, :]

        zr0 = wh[:, ZR0:ZR0 + 128]
        zwb0 = bfview(zr0)
        brt = wh[:, BRT:BRT + 128]
        ecs = [bfview(wh[:, ECS + 2 * c:ECS + 2 * c + 2]) for c in range(NTHI)]
        idn = wh[0:32, IDN:IDN + 32]
        wfc = wh[:, WFC:WFC + 4].bitcast(F32)
        oh = wh[0:4, OH:OH + 4]
        vv = wh[0:32, VV:VV + 2]
        ecs2 = bfview(wh[0:96, ECS2:ECS2 + 2])
        embX = [embs0[:], embs1[:], embs2[:]]
        cv0a = wm[:, 0:128]
        cv0b = wm[:, 128:256]
        zr1 = wm[:, 256:384]
        zwb1 = bfview(zr1)
        cv1a = wm2[:, 0:128]
        cv1b = wm2[:, 128:256]
        zwr2bd = wm2[0:96, 256:352]
        zwb2bd = bfview(zwr2bd)
        wa = wm2[0:32, 352:384]
        zwr2pad = [wm3[0:32, 96 * c:96 * (c + 1)] for c in range(NTHI)]
        cv2 = wm3[0:96, 288:320]
        brt2 = wm3[0:1, 320:416]
        cva = (cv0a, cv1a)
        cvb = (cv0b, cv1b)
        zwb = (zwb0, zwb1)
        zrr = (zr0, zr1)

        # ---------------- warmups + gather indices ----------------
        dumin = spool.tile([128, 2], F32)
        nc.vector.memset(dumin[:], 0.0)
        duout = spool.tile([128, 2], F32)
        nc.scalar.activation(duout[:], dumin[:], AF.Sigmoid)
        dumm = spool.tile([128, 4], F32R)
        nc.vector.memset(dumm[:].bitcast(F32), 0.0)
        pwarm = ppool_s.tile([2, 2], F32, tag="sm", name="pwarm", bufs=1)
        nc.tensor.matmul(pwarm[:], dumm[:, 0:2], dumm[:, 2:4])

        # gather index table: position i = 16*s + (p%16); value = eg row
        # number: c1 rows 0:128, c2 rows 128:256, l2 rows 256:352. One iota
        # gives 128c+16s; the p%16 term is added from a tiny iota column.
        # high_priority: the idx chain gates the gather channel; keep it
        # ahead of the big zeros memset on the DVE queue.
        with tc.high_priority():
            nc.gpsimd.iota(pcol[:, 0:1], pattern=[[0, 1]], base=0,
                           channel_multiplier=1)
            nc.gpsimd.iota(idxs[:, 0:24].rearrange("p (c s) -> p c s", c=3),
                           pattern=[[128, 3], [16, 8]], base=0,
                           channel_multiplier=0)
            nc.vector.tensor_scalar(pcol[:, 1:2], pcol[:, 0:1], 15, None,
                                    ALU.bitwise_and)
            nc.vector.tensor_copy(pcolf[:], pcol[:, 1:2])
            nc.vector.tensor_scalar(idxs[:, 0:24], idxs[:, 0:24], pcolf[:],
                                    None, ALU.add)

        # ---------------- input DMAs ----------------
        # HWDGE slots: head, emb c0, then weights in need order
        nc.sync.dma_start(out=wh[:], in_=wh_d)
        nc.sync.dma_start(out=embs0[:], in_=e0_d)
        # gather channel: emb c1, c2, l2-packed rows
        nc.gpsimd.dma_gather(embs1[:].rearrange("p (c e) -> p c e", c=1),
                             eg_d[:], idxs[:, 0:8], 128, 128, 512)
        nc.gpsimd.dma_gather(embs2[:].rearrange("p (c e) -> p c e", c=1),
                             eg_d[:], idxs[:, 8:16], 128, 128, 512)
        nc.gpsimd.dma_gather(embl2[:].rearrange("p (c e) -> p c e", c=1),
                             eg_d[:], idxs[:, 16:22], 96, 96, 512)
        nc.sync.dma_start(out=wm[:], in_=wm_d)
        nc.sync.dma_start(out=wm2[:], in_=wm2_d)
        nc.sync.dma_start(out=wm3[:], in_=wm3_d)

        for q in range(4):
            nc.vector.memset(zeros[:, 128 * q:128 * (q + 1)], 0.0)
        for l in range(2):
            for c in range(NTHI):
                nc.vector.memset(rs[l][c][:, N + 1:N + 2].bitcast(F32), 0.0)

        # ---------------- helpers ----------------
        def svt_chunk(c):
            win = s_cat[0:32, 384 * c:384 * (c + 1)].rearrange(
                "p (i r) -> p r i", r=3)
            for j in range(3):
                nc.tensor.matmul(psvT[:, c, 2 * j:2 * j + 2],
                                 win[:, j, :], vv)

        def softmax_half(h):
            sl = slice(h, h + 2)
            nc.scalar.activation(u_sb[:, sl, :], psvT[:, sl, :], AF.Tanh,
                                 scale=0.5)
            nc.vector.tensor_scalar(un[:, sl, :], u_sb[:, sl, :], -1.0, 1.0,
                                    ALU.mult, ALU.add)
            nc.vector.reciprocal(un[:, sl, :], un[:, sl, :])
            nc.vector.tensor_scalar(u_sb[:, sl, :], u_sb[:, sl, :], 1.0,
                                    None, ALU.add)
            nc.vector.tensor_tensor(e_sb[:, sl, :], u_sb[:, sl, :],
                                    un[:, sl, :], ALU.mult)
            nc.vector.tensor_reduce(es[:, sl], e_sb[:, sl, :], axis=AX.X,
                                    op=ALU.add)
            nc.vector.tensor_scalar(es[:, sl], es[:, sl], -3.0, None, ALU.add)
            nc.vector.reciprocal(ri[:, sl], es[:, sl])

        def attn_mm(l, rsl2):
            ps = ppool_z.tile([32, N], F32, tag="zb", name=f"ps{l}")
            nc.tensor.matmul(ps[:], wa, rsl2[0:32, 0:N])
            for c in range(NCHUNK):
                nc.tensor.matmul(prt[:, c, l, :],
                                 rsl2[0:32, 128 * c:128 * (c + 1)], idn)
            return ps

        def attn_tanh(l, ps):
            nc.scalar.activation(s_cat[:, l * N:(l + 1) * N], ps[:], AF.Tanh,
                                 bias=wfc[0:32, 3:4], scale=1.0)

        def prt_copy(l):
            nc.vector.tensor_copy(prt_sb[:, :, l, :], prt[:, :, l, :])

        def mix_dve(c):
            ta = tmps[2] if c == 3 else tmps[c]
            nc.vector.tensor_scalar(ta[:], prt_sb[:, c, 0, :], e_sb[:, c, 0:1],
                                    None, ALU.mult)
            nc.vector.scalar_tensor_tensor(ta[:], prt_sb[:, c, 1, :],
                                           e_sb[:, c, 2:3], ta[:],
                                           ALU.mult, ALU.add)
            nc.vector.scalar_tensor_tensor(ta[:], prt_sb[:, c, 2, :],
                                           e_sb[:, c, 4:5], ta[:],
                                           ALU.mult, ALU.add)
            nc.vector.tensor_scalar(y_sb[:, c, :], ta[:], ri[:, c:c + 1], None,
                                    ALU.mult)

        def mix_pool(c):
            ta, tb = tmps[0], tmps[1]
            nc.gpsimd.tensor_scalar(ta[:], prt_sb[:, c, 0, :], e_sb[:, c, 0:1],
                                    None, ALU.mult)
            nc.gpsimd.tensor_scalar(tb[:], prt_sb[:, c, 1, :], e_sb[:, c, 2:3],
                                    None, ALU.mult)
            nc.gpsimd.tensor_tensor(ta[:], ta[:], tb[:], ALU.add)
            nc.gpsimd.tensor_scalar(tb[:], prt_sb[:, c, 2, :], e_sb[:, c, 4:5],
                                    None, ALU.mult)
            nc.gpsimd.tensor_tensor(ta[:], ta[:], tb[:], ALU.add)
            nc.gpsimd.tensor_scalar(y_sb[:, c, :], ta[:], ri[:, c:c + 1], None,
                                    ALU.mult)

        # ------------- layers 0..2, readiness-ordered emission -------------
        # Per-queue instruction order matters (in-order SEQs block on the
        # head's waits), so ops are emitted roughly in dependency-readiness
        # order rather than layer-by-layer.
        psz0 = ppool_s.tile([128, 2 * NTHI], F32, tag="sm", name="psz0",
                            bufs=1)
        psz1 = ppool_s.tile([128, 2 * NTHI], F32, tag="sm", name="psz1",
                            bufs=1)
        pz0 = [None] * NTHI
        pz1 = [None] * NTHI
        pcs = {}

        def emit_pz0(c):
            pz0[c] = ppool_z.tile([128, N], F32, tag="zb", name=f"pz0{c}")
            nc.tensor.matmul(pz0[c][:], zwb0, embX[c], start=True, stop=True)
            nc.tensor.matmul(psz0[:, 2 * c:2 * c + 2], zwb0, ecs[c],
                             start=True, stop=False)
            nc.tensor.matmul(psz0[:, 2 * c:2 * c + 2], brt[0:3, 0:128],
                             oh[0:3, 0:2], start=False, stop=True)
            nc.vector.tensor_copy(bvs[0][:, c:c + 1], psz0[:, 2 * c:2 * c + 1])

        def emit_pz1_emb(c):
            pz1[c] = ppool_z.tile([128, N], F32, tag="zb", name=f"pz1{c}")
            nc.tensor.matmul(pz1[c][:], zwb1, embX[c], start=True, stop=False)

        def emit_pz1_r(c):
            nc.tensor.matmul(pz1[c][:], zr1, rs[0][c][:, 0:N],
                             start=False, stop=True)
            nc.tensor.matmul(psz1[:, 2 * c:2 * c + 2], zwb1, ecs[c],
                             start=True, stop=False)
            nc.tensor.matmul(psz1[:, 2 * c:2 * c + 2], zr1,
                             rs[0][c][:, N:N + 2], start=False, stop=False)
            nc.tensor.matmul(psz1[:, 2 * c:2 * c + 2], brt[0:3, 0:128],
                             oh[0:3, 1:3], start=False, stop=True)
            nc.vector.tensor_copy(bvs[1][:, c:c + 1], psz1[:, 2 * c:2 * c + 1])

        def emit_conv_b(l, c):
            pcs[(l, c)] = ppool_c.tile([128, N], F32, tag="cv",
                                       name=f"pc{l}{c}")
            nc.tensor.matmul(pcs[(l, c)][:], cvb[l], gps[l][c - 1][:],
                             start=True, stop=False)

        def emit_conv_a(l, c):
            if c == 0:
                pcs[(l, c)] = ppool_c.tile([128, N], F32, tag="cv",
                                           name=f"pc{l}{c}")
            nc.tensor.matmul(pcs[(l, c)][:], cva[l], gps[l][c][:],
                             start=(c == 0), stop=True)

        def emit_sig(l, c):
            nc.scalar.activation(gps[l][c][:], (pz0 if l == 0 else pz1)[c][:],
                                 AF.Sigmoid, bias=bvs[l][:, c:c + 1],
                                 scale=1.0)

        def emit_relu(l, c):
            nc.vector.scalar_tensor_tensor(
                rs[l][c][:, 0:N], pcs[(l, c)][:], wfc[:, l:l + 1], zeros[:],
                ALU.add, ALU.max, accum_out=rs[l][c][:, N:N + 1])

        # --- emission schedule ---
        emit_pz0(0)
        emit_pz1_emb(0)
        emit_sig(0, 0)
        emit_conv_a(0, 0)
        emit_conv_b(0, 1)
        emit_pz0(1)
        emit_sig(0, 1)
        emit_relu(0, 0)
        with tc.high_priority(offset=30):
            emit_pz1_r(0)
        emit_conv_a(0, 1)
        emit_conv_b(0, 2)
        emit_pz0(2)
        emit_sig(0, 2)
        emit_relu(0, 1)
        emit_pz1_emb(1)
        emit_pz1_r(1)
        emit_sig(1, 0)
        emit_conv_a(0, 2)
        emit_relu(0, 2)
        emit_pz1_emb(2)
        emit_pz1_r(2)
        emit_sig(1, 1)
        emit_sig(1, 2)

        # layer-2 packed pz + conv1 interleaved by readiness
        psz2 = ppool_s.tile([96, 2], F32, tag="sm", name="psz2", bufs=1)
        pz2 = ppool_c.tile([96, N], F32, tag="cv", name="pz2")
        nc.tensor.matmul(pz2[:], zwb2bd, embl2[0:96, :], start=True, stop=False)
        nc.tensor.matmul(psz2[:], zwb2bd, ecs2, start=True, stop=False)

        emit_conv_a(1, 0)
        emit_conv_b(1, 1)
        emit_relu(1, 0)

        def emit_pz2_r(j, c, last=False):
            nc.tensor.matmul(pz2[:], zwr2pad[c], rs[j][c][0:32, 0:N],
                             start=False, stop=last)
            nc.tensor.matmul(psz2[:], zwr2pad[c], rs[j][c][0:32, N:N + 2],
                             start=False, stop=False)

        emit_pz2_r(0, 0)
        emit_conv_a(1, 1)
        emit_conv_b(1, 2)
        emit_pz2_r(0, 1)
        emit_pz2_r(1, 0)
        ps0 = attn_mm(0, rs[0][2])
        attn_tanh(0, ps0)
        with tc.high_priority(offset=20):
            emit_conv_a(1, 2)
        # r1c1 runs on ACT (its window before g2 is idle); DVE carries r1c2
        # immediately after its conv so pz2 closes sooner.
        nc.scalar.activation(rs[1][1][:, 0:N], pcs[(1, 1)][:], AF.Relu,
                             bias=wfc[:, 1:2], scale=1.0,
                             accum_out=rs[1][1][:, N:N + 1])
        with tc.high_priority(offset=20):
            emit_relu(1, 2)
            emit_pz2_r(0, 2)
            nc.tensor.matmul(psz2[:], brt2, oh[0:1, 0:2], start=False,
                             stop=False)
            emit_pz2_r(1, 1)
            emit_pz2_r(1, 2, last=True)
            nc.vector.tensor_copy(bv2[:], psz2[:, 0:1])
        prt_copy(0)
        nc.scalar.activation(g2all[:], pz2[:], AF.Sigmoid,
                             bias=bv2[:], scale=1.0)
        ps1 = attn_mm(1, rs[1][2])
        attn_tanh(1, ps1)
        prt_copy(1)
        svt_chunk(0)
        svt_chunk(1)
        softmax_half(0)

        pc2 = ppool_c.tile([32, N], F32, tag="cv", name="pc2")
        nc.tensor.matmul(pc2[:], cv2, g2all[:], start=True, stop=True)
        nc.vector.tensor_scalar(rs2[:], pc2[:], wfc[0:32, 2:3], 0.0,
                                ALU.add, ALU.max)
        ps2 = attn_mm(2, rs2)
        attn_tanh(2, ps2)
        prt_copy(2)
        svt_chunk(2)
        svt_chunk(3)

        # chunks 0/1 need only softmax half 0 + the (now complete) prt_sb
        mix_dve(0)
        mix_dve(1)
        y_view = y_d.rearrange("(c p) f -> p c f", p=128)
        nc.sync.dma_start(out=y_view[:, 0:2, :], in_=y_sb[:, 0:2, :])

        softmax_half(2)
        mix_pool(2)
        mix_dve(3)
        nc.sync.dma_start(out=y_view[:, 2:4, :], in_=y_sb[:, 2:4, :])
        if debug:
            cvt = spool.tile([128, 512], F32, name="cvt")
            def dump(nm, ap):
                nc.gpsimd.dma_start(out=dbg[nm], in_=ap)
            nc.vector.tensor_copy(cvt[:, 0:512], embs0[:])
            dump("emb0", cvt[:, 0:512])
            cvt2 = spool.tile([128, 512], F32, name="cvt2")
            nc.vector.tensor_copy(cvt2[:, 0:512], embs1[:])
            dump("embs1", cvt2[:, 0:512])
            cvt3 = spool.tile([128, 512], F32, name="cvt3")
            nc.vector.tensor_copy(cvt3[:, 0:512], embl2[:])
            dump("embl2", cvt3[:, 0:512])
            cvt4 = spool.tile([128, 128], F32, name="cvt4")
            nc.vector.tensor_copy(cvt4[:], zwb0)
            dump("zwb0", cvt4[:])
            dump("cv0a", cv0a.bitcast(F32))
            dump("g00", gps[0][0][:].bitcast(F32))
            dump("r00", rs[0][0][:].bitcast(F32))
            dump("bv0", bvs[0][:])
            dump("g2", g2all[:].bitcast(F32))
            dump("g10", gps[1][0][:].bitcast(F32))
            dump("r10", rs[1][0][:].bitcast(F32))
            dump("r12", rs[1][2][:].bitcast(F32))
            dump("bv2d", bv2[:])
            dump("rs2", rs2[:].bitcast(F32))
            dump("scat", s_cat[:].bitcast(F32))
            dump("esb", e_sb[:].rearrange("p a b -> p (a b)"))
            dump("prt", prt_sb[:].rearrange("p a b c -> p (a b c)"))
            dump("idxs", idxs[:])

    nc.finalize()
    _prune_redundant_act_loads(nc)
    return nc


def _host_weights(Wd, bd, Ws, bs, Wg, bg, Wc, bc, Wa, ba, v, emb_perm):
    """Build wh/wm/wm2/wm3 (plain f32 arrays for the f32r tensors) and the
    eg gather rows (bf16) for one batch element."""
    f32 = np.float32
    bf = ml_dtypes.bfloat16
    dinv = f32(1.0) / np.sqrt(f32(513.0))
    c2 = f32(dinv * dinv)

    Mcs, brs = [], []
    for l in range(L):
        M = (Ws[l] @ Wg[:FD] + Wd[l] @ Wg[FD:]).astype(f32)
        Mcs.append((c2 * M).astype(f32))
        brs.append((bs[l] @ Wg[:FD] + bd[l] @ Wg[FD:] + bg).astype(f32))

    def blockdiag(M, nblk, rows=128):
        out = np.zeros((rows, 32 * nblk), f32)
        for q in range(nblk):
            out[32 * q:32 * (q + 1), 32 * q:32 * (q + 1)] = M
        return out

    def conv_merged(l):
        A = np.zeros((128, 128), f32)
        B = np.zeros((128, 128), f32)
        for (q_in, q_out, k, carry) in _conv_cells(l):
            blk = Wc[l][:, :, 0, k].T.astype(f32)
            dst = A if carry == 0 else B
            assert carry in (0, -1)
            assert not dst[32 * q_in:32 * (q_in + 1),
                           32 * q_out:32 * (q_out + 1)].any()
            dst[32 * q_in:32 * (q_in + 1), 32 * q_out:32 * (q_out + 1)] = blk
        return A, B

    embc = []
    for c in range(NTHI):
        tile_c = np.zeros((128, N), f32)
        for p in range(128):
            t = 4 * c + (3 - p // 32)
            tile_c[p] = emb_perm[t, p % 32]
        embc.append(tile_c)
    # colsum of the bf16-rounded tiles (what the device matmuls actually see)
    ecs_host = [e.astype(bf).astype(f32).sum(axis=1) for e in embc]

    wh = np.zeros((128, WH_COLS), f32)
    wh[:, ZR0:ZR0 + 128] = blockdiag(Mcs[0], 4)
    for l in range(L):
        wh[l, BRT:BRT + 128] = np.tile(brs[l], 4)
    for c in range(NTHI):
        wh[:, ECS + 2 * c] = ecs_host[c]
    wh[0:32, IDN:IDN + 32] = np.eye(32, dtype=f32)
    wfc_host = np.zeros((128, 4), f32)
    for l in range(L):
        wfc_host[:, l] = np.tile(bc[l].astype(f32), 4)
    wfc_host[0:32, 3] = ba.astype(f32)
    wh[:, WFC:WFC + 4] = wfc_host
    for l in range(L):
        wh[l, OH + l] = 1.0
    wh[0:32, VV] = v[:, 0].astype(f32)
    for c in range(NTHI):
        wh[32 * c:32 * (c + 1), ECS2] = ecs_host[c][0:32]

    wm = np.zeros((128, WM_COLS), f32)
    A0, B0 = conv_merged(0)
    wm[:, 0:128] = A0
    wm[:, 128:256] = B0
    wm[:, 256:384] = blockdiag(Mcs[1], 4)

    wm2 = np.zeros((128, WM2_COLS), f32)
    A1, B1 = conv_merged(1)
    wm2[:, 0:128] = A1
    wm2[:, 128:256] = B1
    wm2[0:96, 256:352] = blockdiag(Mcs[2], 3, rows=96)
    wm2[0:32, 352:384] = Wa.astype(f32)

    wm3 = np.zeros((128, WM3_COLS), f32)
    for c in range(NTHI):
        wm3[0:32, 96 * c + 32 * c:96 * c + 32 * (c + 1)] = Mcs[2]
    for k in range(K):
        wm3[32 * k:32 * (k + 1), 288:320] = Wc[2][:, :, 0, k].T
    wm3[0, 320:416] = np.tile(brs[2], 3)

    eg = np.zeros((352, 512), bf)
    eg[0:128] = embc[1].astype(bf)
    eg[128:256] = embc[2].astype(bf)
    for c in range(NTHI):
        eg[256 + 32 * c:256 + 32 * (c + 1)] = embc[c][0:32].astype(bf)
    e0 = embc[0].astype(bf)
    return wh, eg, wm, wm2, wm3, e0


def kernel(**inputs):
    node_embeddings = np.asarray(inputs["node_embeddings"], dtype=np.float32)
    args = tuple(np.asarray(inputs[k], np.float32) for k in
                 ("Wd", "bd", "Ws", "bs", "Wg", "bg", "Wc", "bc", "Wa", "ba", "v"))

    if "nc" not in _CACHE:
        _CACHE["nc"] = _build_nc()
    nc = _CACHE["nc"]

    n_cores = 8
    in_maps = []
    for i in range(n_cores):
        wh, eg, wm, wm2, wm3, e0 = _host_weights(*args,
                                                 node_embeddings[i % BSZ])
        in_maps.append({"wh": wh, "eg": eg, "wm": wm, "wm2": wm2, "wm3": wm3,
                        "e0": e0})
    res = run_bass_kernel_spmd(nc, in_maps, core_ids=list(range(n_cores)))
    y = np.stack([res.results[b]["y"] for b in range(BSZ)], axis=0)
    return y.astype(np.float32)


# revision 3
# speedup vs baseline: 1.0066x; 1.0066x over previous
"""Trainium2 Bass kernel for nn_DilatedSpatioTemporalGCN — v3.

Same math as v2 (adjacency collapses to (J+I)/513; per-layer
Z = c2*(X + 1 colsum(X)^T) @ M_l + bias_row, g = sigmoid(Z),
r = relu(dilated causal conv(g) + bc), X += r; tiny 3-way attention tail),
with a restructured schedule:

- emb tiles stay bf16 and feed matmuls directly (no upconverts); Z weights
  are duplicated: bf16 copies multiply emb, f32r copies multiply the f32r
  residual r tiles. Host precomputes the emb colsum columns.
- conv taps are merged by chunk-carry: all (k, q_in->q_out) cells for a
  given carry land in one 128x128 lhsT, so a chunk's conv is 2 matmuls.
- layer 2 is fully packed: chunk c lives on partitions 32c:32c+32 of one
  [96,512] PSUM tile via block-diagonal / zero-padded lhsT weights (dst
  partition base stays 0), one sigmoid, one K-stacked conv matmul.
- inputs: one HWDGE DMA carries the weight head + emb c0; emb c1/c2 and
  the packed l2 rows arrive via gpsimd dma_gather (parallel SWDGE channel);
  remaining weights stream on later HWDGE slots in need order.

Sharding: data-parallel over batch (4 elements) on cores 0-3; cores 4-7
run duplicate batches (harmless, keeps all 8 cores uniform).
"""

import os
import numpy as np
import ml_dtypes
from contextlib import ExitStack

import concourse.bacc as bacc
import concourse.tile as tile
from concourse import mybir
from concourse.bass_utils import run_bass_kernel_spmd

F32 = mybir.dt.float32
F32R = mybir.dt.float32r
BF16 = mybir.dt.bfloat16
I16 = mybir.dt.int16
AF = mybir.ActivationFunctionType
ALU = mybir.AluOpType
AX = mybir.AxisListType

BSZ, T, FD, N = 4, 12, 32, 512
L, K = 3, 3
DIL = (1, 2, 4)
NTHI = 3
NCHUNK = 4

# All weight containers are plain F32R dram tensors (host f32 values; the
# runtime rounds to f32r's ~12-bit mantissa; memory layout is IEEE f32
# bytes). bf16 operands for the emb-side matmuls are STRIDE-2 bitcast views
# picking the high u16 of each f32r value (== bf16 truncation). Offsets are
# f32r columns.
ZR0 = 0             # 128: blockdiag Mc0 f32r (zwb0 = bf16 view of it)
BRT = 128           # 128: [3 rows, 128] f32r tiled bias rows per layer
ECS = 256           # 6: emb colsums per chunk ([128,2] f32r, col1 zero)
IDN = 262           # 32: [32,32] f32r identity
WFC = 294           # 4: [128,4] f32 bitcast: bc0..2 tiled; ba (rows 0:32)
OH = 298            # 4: [4,4] f32r one-hot cols
VV = 302            # 2: [32,2] f32r v (col1 zero)
ECS2 = 304          # 2: [96,2] f32r packed l2 colsums (col1 zero)
WH_COLS = 306

# wm (slot2): CV0A 0:128, CV0B 128:256, ZR1 256:384 (blockdiag Mc1)
WM_COLS = 384
# wm2 (slot3): CV1A 0:128, CV1B 128:256, ZWR2BD 256:352 ([96,96] blockdiag
#   Mc2; zwb2bd = bf16 view), WA 352:384
WM2_COLS = 384
# wm3 (slot4): ZWR2PAD 0:288 ([32,96] x3), CV2 288:320 ([96,32]),
#   BRT2 320:416 ([1,96])
WM3_COLS = 416

_CACHE = {}


def _conv_cells(l):
    """Cells (q_in, q_out, k, carry) for layer l in the reversed-group
    layout; carry in {0,-1}."""
    d = DIL[l]
    cells = []
    for k in range(K):
        delta = (K - 1 - k) * d
        for q_out in range(4):
            a = 3 - q_out
            ap_ = (a - delta) % 4
            carry = (a - delta - ap_) // 4
            cells.append((3 - ap_, q_out, k, carry))
    return cells


def _prune_redundant_act_loads(nc):
    for blk in nc.main_func.blocks:
        insts = blk.instructions
        dead = []
        prev_load = None
        for inst in insts:
            if isinstance(inst, mybir.InstLoadActFuncSet):
                if prev_load is not None:
                    dead.append(prev_load)
                prev_load = inst
            elif isinstance(inst, mybir.InstActivation):
                prev_load = None
        for inst in dead:
            si = getattr(inst, "sync_info", None)
            if si is not None and (len(si.on_wait) > 0 or len(si.on_update) > 0):
                continue
            insts.remove(inst)


def _build_nc():
    nc = bacc.Bacc("TRN2", target_bir_lowering=False, debug=False)
    wh_d = nc.dram_tensor("wh", [128, WH_COLS], F32R, kind="ExternalInput").ap()
    eg_d = nc.dram_tensor("eg", [352, 512], BF16, kind="ExternalInput").ap()
    e0_d = nc.dram_tensor("e0", [128, 512], BF16, kind="ExternalInput").ap()
    wm_d = nc.dram_tensor("wm", [128, WM_COLS], F32R, kind="ExternalInput").ap()
    wm2_d = nc.dram_tensor("wm2", [128, WM2_COLS], F32R, kind="ExternalInput").ap()
    wm3_d = nc.dram_tensor("wm3", [128, WM3_COLS], F32R, kind="ExternalInput").ap()
    y_d = nc.dram_tensor("y", [N, FD], F32, kind="ExternalOutput").ap()
    debug = bool(int(os.environ.get("K_DEBUG", "0")))
    if debug:
        dbg = {}
        for nm, shape in [("emb0", [128, 512]), ("embs1", [128, 512]),
                          ("embl2", [128, 512]), ("g00", [128, 512]),
                          ("r00", [128, 514]), ("bv0", [128, 3]),
                          ("g2", [96, 512]), ("rs2", [32, 512]),
                          ("g10", [128, 512]), ("r10", [128, 514]),
                          ("r12", [128, 514]), ("bv2d", [96, 1]),
                          ("scat", [32, 1536]), ("esb", [128, 24]),
                          ("prt", [128, 384]), ("idxs", [128, 32]),
                          ("zwb0", [128, 128]), ("cv0a", [128, 128])]:
            dt = I16 if nm == "idxs" else F32
            dbg[nm] = nc.dram_tensor("dbg_" + nm, shape, dt,
                                     kind="ExternalOutput").ap()

    with tile.TileContext(nc) as tc, ExitStack() as ctx:
        ctx.enter_context(nc.allow_low_precision(
            "bf16 emb path validated to 3e-4 scale error; f32r elsewhere"))
        wpool = ctx.enter_context(tc.tile_pool(name="w", bufs=1))
        spool = ctx.enter_context(tc.tile_pool(name="s", bufs=1))
        ppool_z = ctx.enter_context(tc.tile_pool(name="pz", bufs=3, space="PSUM"))
        ppool_c = ctx.enter_context(tc.tile_pool(name="pc", bufs=3, space="PSUM"))
        ppool_s = ctx.enter_context(tc.tile_pool(name="psm", bufs=2, space="PSUM"))
        ppool_k = ctx.enter_context(tc.tile_pool(name="pk", bufs=1, space="PSUM"))

        # ---------------- persistent tiles ----------------
        wh = wpool.tile([128, WH_COLS], F32R)
        wm = wpool.tile([128, WM_COLS], F32R)
        wm2 = wpool.tile([128, WM2_COLS], F32R)
        wm3 = wpool.tile([128, WM3_COLS], F32R)
        embs0 = wpool.tile([128, 512], BF16, name="embs0")
        embs1 = wpool.tile([128, 512], BF16, name="embs1")
        embs2 = wpool.tile([128, 512], BF16, name="embs2")
        embl2 = wpool.tile([128, 512], BF16, name="embl2")
        idxs = spool.tile([128, 24], I16, name="idxs")
        pcol = spool.tile([128, 2], I16, name="pcol")
        pcolf = spool.tile([128, 1], F32, name="pcolf")
        zeros = spool.tile([128, N], F32, name="zeros")

        gps = [[wpool.tile([128, N], F32R, name=f"g{l}{c}") for c in range(NTHI)]
               for l in range(2)]
        g2all = wpool.tile([96, N], F32R, name="g2all")
        rs = [[wpool.tile([128, N + 2], F32R, name=f"r{l}{c}") for c in range(NTHI)]
              for l in range(2)]
        rs2 = wpool.tile([32, N], F32R, name="rs2")
        bvs = [spool.tile([128, NTHI], F32, name=f"bv{l}") for l in range(2)]
        bv2 = spool.tile([96, 1], F32, name="bv2")
        s_cat = spool.tile([32, L * N], F32R, name="s_cat")
        e_sb = spool.tile([128, NCHUNK, 6], F32)
        es = spool.tile([128, NCHUNK], F32)
        ri = spool.tile([128, NCHUNK], F32)
        y_sb = spool.tile([128, NCHUNK, FD], F32)
        tmps = [spool.tile([128, FD], F32, name=f"ytmp{i}") for i in range(3)]
        u_sb = spool.tile([128, NCHUNK, 6], F32, name="u_sb")
        un = spool.tile([128, NCHUNK, 6], F32, name="un")
        prt_sb = spool.tile([128, NCHUNK, L, FD], F32, name="prt_sb")
        pkt = ppool_k.tile([128, NCHUNK * L * FD + NCHUNK * 6], F32,
                           tag="prt", name="pkt")
        prt = pkt[:, 0:NCHUNK * L * FD].rearrange(
            "p (c l f) -> p c l f", c=NCHUNK, l=L)
        psvT = pkt[:, NCHUNK * L * FD:].rearrange(
            "p (c s) -> p c s", c=NCHUNK)

        # weight slices; bf16 operands are stride-2 (high-u16) views
        def bfview(region):
            return region.bitcast(BF16).rearrange(
                "p (c two) -> p two c", two=2)[:, 1, :]

        zr0 = wh[:, ZR0:ZR0 + 128]
        zwb0 = bfview(zr0)
        brt = wh[:, BRT:BRT + 128]
        ecs = [bfview(wh[:, ECS + 2 * c:ECS + 2 * c + 2]) for c in range(NTHI)]
        idn = wh[0:32, IDN:IDN + 32]
        wfc = wh[:, WFC:WFC + 4].bitcast(F32)
        oh = wh[0:4, OH:OH + 4]
        vv = wh[0:32, VV:VV + 2]
        ecs2 = bfview(wh[0:96, ECS2:ECS2 + 2])
        embX = [embs0[:], embs1[:], embs2[:]]
        cv0a = wm[:, 0:128]
        cv0b = wm[:, 128:256]
        zr1 = wm[:, 256:384]
        zwb1 = bfview(zr1)
        cv1a = wm2[:, 0:128]
        cv1b = wm2[:, 128:256]
        zwr2bd = wm2[0:96, 256:352]
        zwb2bd = bfview(zwr2bd)
        wa = wm2[0:32, 352:384]
        zwr2pad = [wm3[0:32, 96 * c:96 * (c + 1)] for c in range(NTHI)]
        cv2 = wm3[0:96, 288:320]
        brt2 = wm3[0:1, 320:416]
        cva = (cv0a, cv1a)
        cvb = (cv0b, cv1b)
        zwb = (zwb0, zwb1)
        zrr = (zr0, zr1)

        # ---------------- warmups + gather indices ----------------
        dumin = spool.tile([128, 2], F32)
        nc.vector.memset(dumin[:], 0.0)
        duout = spool.tile([128, 2], F32)
        nc.scalar.activation(duout[:], dumin[:], AF.Sigmoid)
        dumm = spool.tile([128, 4], F32R)
        nc.vector.memset(dumm[:].bitcast(F32), 0.0)
        pwarm = ppool_s.tile([2, 2], F32, tag="sm", name="pwarm", bufs=1)
        nc.tensor.matmul(pwarm[:], dumm[:, 0:2], dumm[:, 2:4])

        # gather index table: position i = 16*s + (p%16); value = eg row
        # number: c1 rows 0:128, c2 rows 128:256, l2 rows 256:352. One iota
        # gives 128c+16s; the p%16 term is added from a tiny iota column.
        # high_priority: the idx chain gates the gather channel; keep it
        # ahead of the big zeros memset on the DVE queue.
        with tc.high_priority():
            nc.gpsimd.iota(pcol[:, 0:1], pattern=[[0, 1]], base=0,
                           channel_multiplier=1)
            nc.gpsimd.iota(idxs[:, 0:24].rearrange("p (c s) -> p c s", c=3),
                           pattern=[[128, 3], [16, 8]], base=0,
                           channel_multiplier=0)
            nc.vector.tensor_scalar(pcol[:, 1:2], pcol[:, 0:1], 15, None,
                                    ALU.bitwise_and)
            nc.vector.tensor_copy(pcolf[:], pcol[:, 1:2])
            nc.vector.tensor_scalar(idxs[:, 0:24], idxs[:, 0:24], pcolf[:],
                                    None, ALU.add)

        # ---------------- input DMAs ----------------
        # HWDGE slots: head, emb c0, then weights in need order
        nc.sync.dma_start(out=wh[:], in_=wh_d)
        nc.sync.dma_start(out=embs0[:], in_=e0_d)
        # gather channel: emb c1, c2, l2-packed rows
        nc.gpsimd.dma_gather(embs1[:].rearrange("p (c e) -> p c e", c=1),
                             eg_d[:], idxs[:, 0:8], 128, 128, 512)
        nc.gpsimd.dma_gather(embs2[:].rearrange("p (c e) -> p c e", c=1),
                             eg_d[:], idxs[:, 8:16], 128, 128, 512)
        nc.gpsimd.dma_gather(embl2[:].rearrange("p (c e) -> p c e", c=1),
                             eg_d[:], idxs[:, 16:22], 96, 96, 512)
        nc.sync.dma_start(out=wm[:], in_=wm_d)
        nc.sync.dma_start(out=wm2[:], in_=wm2_d)
        nc.sync.dma_start(out=wm3[:], in_=wm3_d)

        for q in range(4):
            nc.vector.memset(zeros[:, 128 * q:128 * (q + 1)], 0.0)
        for l in range(2):
            for c in range(NTHI):
                nc.vector.memset(rs[l][c][:, N + 1:N + 2].bitcast(F32), 0.0)

        # ---------------- helpers ----------------
        def svt_chunk(c):
            win = s_cat[0:32, 384 * c:384 * (c + 1)].rearrange(
                "p (i r) -> p r i", r=3)
            for j in range(3):
                nc.tensor.matmul(psvT[:, c, 2 * j:2 * j + 2],
                                 win[:, j, :], vv)

        def softmax_half(h):
            sl = slice(h, h + 2)
            nc.scalar.activation(u_sb[:, sl, :], psvT[:, sl, :], AF.Tanh,
                                 scale=0.5)
            nc.vector.tensor_scalar(un[:, sl, :], u_sb[:, sl, :], -1.0, 1.0,
                                    ALU.mult, ALU.add)
            nc.vector.reciprocal(un[:, sl, :], un[:, sl, :])
            nc.vector.tensor_scalar(u_sb[:, sl, :], u_sb[:, sl, :], 1.0,
                                    None, ALU.add)
            nc.vector.tensor_tensor(e_sb[:, sl, :], u_sb[:, sl, :],
                                    un[:, sl, :], ALU.mult)
            nc.vector.tensor_reduce(es[:, sl], e_sb[:, sl, :], axis=AX.X,
                                    op=ALU.add)
            nc.vector.tensor_scalar(es[:, sl], es[:, sl], -3.0, None, ALU.add)
            nc.vector.reciprocal(ri[:, sl], es[:, sl])

        def attn_mm(l, rsl2):
            ps = ppool_z.tile([32, N], F32, tag="zb", name=f"ps{l}")
            nc.tensor.matmul(ps[:], wa, rsl2[0:32, 0:N])
            for c in range(NCHUNK):
                nc.tensor.matmul(prt[:, c, l, :],
                                 rsl2[0:32, 128 * c:128 * (c + 1)], idn)
            return ps

        def attn_tanh(l, ps):
            nc.scalar.activation(s_cat[:, l * N:(l + 1) * N], ps[:], AF.Tanh,
                                 bias=wfc[0:32, 3:4], scale=1.0)

        def prt_copy(l):
            nc.vector.tensor_copy(prt_sb[:, :, l, :], prt[:, :, l, :])

        def mix_dve(c):
            ta = tmps[2] if c == 3 else tmps[c]
            nc.vector.tensor_scalar(ta[:], prt_sb[:, c, 0, :], e_sb[:, c, 0:1],
                                    None, ALU.mult)
            nc.vector.scalar_tensor_tensor(ta[:], prt_sb[:, c, 1, :],
                                           e_sb[:, c, 2:3], ta[:],
                                           ALU.mult, ALU.add)
            nc.vector.scalar_tensor_tensor(ta[:], prt_sb[:, c, 2, :],
                                           e_sb[:, c, 4:5], ta[:],
                                           ALU.mult, ALU.add)
            nc.vector.tensor_scalar(y_sb[:, c, :], ta[:], ri[:, c:c + 1], None,
                                    ALU.mult)

        def mix_pool(c):
            ta, tb = tmps[0], tmps[1]
            nc.gpsimd.tensor_scalar(ta[:], prt_sb[:, c, 0, :], e_sb[:, c, 0:1],
                                    None, ALU.mult)
            nc.gpsimd.tensor_scalar(tb[:], prt_sb[:, c, 1, :], e_sb[:, c, 2:3],
                                    None, ALU.mult)
            nc.gpsimd.tensor_tensor(ta[:], ta[:], tb[:], ALU.add)
            nc.gpsimd.tensor_scalar(tb[:], prt_sb[:, c, 2, :], e_sb[:, c, 4:5],
                                    None, ALU.mult)
            nc.gpsimd.tensor_tensor(ta[:], ta[:], tb[:], ALU.add)
            nc.gpsimd.tensor_scalar(y_sb[:, c, :], ta[:], ri[:, c:c + 1], None,
                                    ALU.mult)

        # ------------- layers 0..2, readiness-ordered emission -------------
        # Per-queue instruction order matters (in-order SEQs block on the
        # head's waits), so ops are emitted roughly in dependency-readiness
        # order rather than layer-by-layer.
        psz0 = ppool_s.tile([128, 2 * NTHI], F32, tag="sm", name="psz0",
                            bufs=1)
        psz1 = ppool_s.tile([128, 2 * NTHI], F32, tag="sm", name="psz1",
                            bufs=1)
        pz0 = [None] * NTHI
        pz1 = [None] * NTHI
        pcs = {}

        def emit_pz0(c):
            pz0[c] = ppool_z.tile([128, N], F32, tag="zb", name=f"pz0{c}")
            nc.tensor.matmul(pz0[c][:], zwb0, embX[c], start=True, stop=True)
            nc.tensor.matmul(psz0[:, 2 * c:2 * c + 2], zwb0, ecs[c],
                             start=True, stop=False)
            nc.tensor.matmul(psz0[:, 2 * c:2 * c + 2], brt[0:3, 0:128],
                             oh[0:3, 0:2], start=False, stop=True)
            nc.vector.tensor_copy(bvs[0][:, c:c + 1], psz0[:, 2 * c:2 * c + 1])

        def emit_pz1_emb(c):
            pz1[c] = ppool_z.tile([128, N], F32, tag="zb", name=f"pz1{c}")
            nc.tensor.matmul(pz1[c][:], zwb1, embX[c], start=True, stop=False)

        def emit_pz1_r(c):
            nc.tensor.matmul(pz1[c][:], zr1, rs[0][c][:, 0:N],
                             start=False, stop=True)
            nc.tensor.matmul(psz1[:, 2 * c:2 * c + 2], zwb1, ecs[c],
                             start=True, stop=False)
            nc.tensor.matmul(psz1[:, 2 * c:2 * c + 2], zr1,
                             rs[0][c][:, N:N + 2], start=False, stop=False)
            nc.tensor.matmul(psz1[:, 2 * c:2 * c + 2], brt[0:3, 0:128],
                             oh[0:3, 1:3], start=False, stop=True)
            nc.vector.tensor_copy(bvs[1][:, c:c + 1], psz1[:, 2 * c:2 * c + 1])

        def emit_conv_b(l, c):
            pcs[(l, c)] = ppool_c.tile([128, N], F32, tag="cv",
                                       name=f"pc{l}{c}")
            nc.tensor.matmul(pcs[(l, c)][:], cvb[l], gps[l][c - 1][:],
                             start=True, stop=False)

        def emit_conv_a(l, c):
            if c == 0:
                pcs[(l, c)] = ppool_c.tile([128, N], F32, tag="cv",
                                           name=f"pc{l}{c}")
            nc.tensor.matmul(pcs[(l, c)][:], cva[l], gps[l][c][:],
                             start=(c == 0), stop=True)

        def emit_sig(l, c):
            nc.scalar.activation(gps[l][c][:], (pz0 if l == 0 else pz1)[c][:],
                                 AF.Sigmoid, bias=bvs[l][:, c:c + 1],
                                 scale=1.0)

        def emit_relu(l, c):
            nc.vector.scalar_tensor_tensor(
                rs[l][c][:, 0:N], pcs[(l, c)][:], wfc[:, l:l + 1], zeros[:],
                ALU.add, ALU.max, accum_out=rs[l][c][:, N:N + 1])

        # --- emission schedule ---
        emit_pz0(0)
        emit_pz1_emb(0)
        emit_sig(0, 0)
        emit_conv_a(0, 0)
        emit_conv_b(0, 1)
        emit_pz0(1)
        emit_sig(0, 1)
        emit_relu(0, 0)
        with tc.high_priority(offset=30):
            emit_pz1_r(0)
        emit_conv_a(0, 1)
        emit_conv_b(0, 2)
        emit_pz0(2)
        emit_sig(0, 2)
        emit_relu(0, 1)
        emit_pz1_emb(1)
        emit_pz1_r(1)
        emit_sig(1, 0)
        emit_conv_a(0, 2)
        emit_relu(0, 2)
        emit_pz1_emb(2)
        emit_pz1_r(2)
        emit_sig(1, 1)
        emit_sig(1, 2)

        # layer-2 packed pz + conv1 interleaved by readiness
        psz2 = ppool_s.tile([96, 2], F32, tag="sm", name="psz2", bufs=1)
        pz2 = ppool_c.tile([96, N], F32, tag="cv", name="pz2")
        nc.tensor.matmul(pz2[:], zwb2bd, embl2[0:96, :], start=True, stop=False)
        nc.tensor.matmul(psz2[:], zwb2bd, ecs2, start=True, stop=False)

        emit_conv_a(1, 0)
        emit_conv_b(1, 1)
        emit_relu(1, 0)

        def emit_pz2_r(j, c, last=False):
            nc.tensor.matmul(pz2[:], zwr2pad[c], rs[j][c][0:32, 0:N],
                             start=False, stop=last)
            nc.tensor.matmul(psz2[:], zwr2pad[c], rs[j][c][0:32, N:N + 2],
                             start=False, stop=False)

        emit_pz2_r(0, 0)
        emit_conv_a(1, 1)
        emit_conv_b(1, 2)
        emit_pz2_r(0, 1)
        emit_pz2_r(1, 0)
        ps0 = attn_mm(0, rs[0][2])
        attn_tanh(0, ps0)
        with tc.high_priority(offset=20):
            emit_conv_a(1, 2)
        # r1c1 runs on ACT (its window before g2 is idle); DVE carries r1c2
        # immediately after its conv so pz2 closes sooner.
        nc.scalar.activation(rs[1][1][:, 0:N], pcs[(1, 1)][:], AF.Relu,
                             bias=wfc[:, 1:2], scale=1.0,
                             accum_out=rs[1][1][:, N:N + 1])
        with tc.high_priority(offset=20):
            emit_relu(1, 2)
            emit_pz2_r(0, 2)
            nc.tensor.matmul(psz2[:], brt2, oh[0:1, 0:2], start=False,
                             stop=False)
            emit_pz2_r(1, 1)
            emit_pz2_r(1, 2, last=True)
            nc.vector.tensor_copy(bv2[:], psz2[:, 0:1])
        prt_copy(0)
        nc.scalar.activation(g2all[:], pz2[:], AF.Sigmoid,
                             bias=bv2[:], scale=1.0)
        ps1 = attn_mm(1, rs[1][2])
        attn_tanh(1, ps1)
        prt_copy(1)
        svt_chunk(0)
        svt_chunk(1)
        softmax_half(0)

        pc2 = ppool_c.tile([32, N], F32, tag="cv", name="pc2")
        nc.tensor.matmul(pc2[:], cv2, g2all[:], start=True, stop=True)
        nc.vector.tensor_scalar(rs2[:], pc2[:], wfc[0:32, 2:3], 0.0,
                                ALU.add, ALU.max)
        ps2 = attn_mm(2, rs2)
        attn_tanh(2, ps2)
        prt_copy(2)
        svt_chunk(2)
        svt_chunk(3)

        # chunks 0/1 need only softmax half 0 + the (now complete) prt_sb
        mix_dve(0)
        mix_dve(1)
        y_view = y_d.rearrange("(c p) f -> p c f", p=128)
        nc.sync.dma_start(out=y_view[:, 0:2, :], in_=y_sb[:, 0:2, :])

        softmax_half(2)
        ta, tb, td = tmps[0], tmps[1], tmps[2]
        nc.gpsimd.tensor_scalar(ta[:], prt_sb[:, 2, 0, :], e_sb[:, 2, 0:1],
                                None, ALU.mult)
        nc.gpsimd.tensor_scalar(tb[:], prt_sb[:, 2, 1, :], e_sb[:, 2, 2:3],
                                None, ALU.mult)
        nc.gpsimd.tensor_tensor(ta[:], ta[:], tb[:], ALU.add)
        nc.gpsimd.tensor_scalar(tb[:], prt_sb[:, 2, 2, :], e_sb[:, 2, 4:5],
                                None, ALU.mult)
        mix_dve(3)
        nc.vector.scalar_tensor_tensor(y_sb[:, 2, :], tb[:], ri[:, 2:3],
                                       ta[:], ALU.bypass, ALU.add)
        nc.vector.tensor_scalar(y_sb[:, 2, :], y_sb[:, 2, :], ri[:, 2:3],
                                None, ALU.mult)
        nc.sync.dma_start(out=y_view[:, 2:4, :], in_=y_sb[:, 2:4, :])
        if debug:
            cvt = spool.tile([128, 512], F32, name="cvt")
            def dump(nm, ap):
                nc.gpsimd.dma_start(out=dbg[nm], in_=ap)
            nc.vector.tensor_copy(cvt[:, 0:512], embs0[:])
            dump("emb0", cvt[:, 0:512])
            cvt2 = spool.tile([128, 512], F32, name="cvt2")
            nc.vector.tensor_copy(cvt2[:, 0:512], embs1[:])
            dump("embs1", cvt2[:, 0:512])
            cvt3 = spool.tile([128, 512], F32, name="cvt3")
            nc.vector.tensor_copy(cvt3[:, 0:512], embl2[:])
            dump("embl2", cvt3[:, 0:512])
            cvt4 = spool.tile([128, 128], F32, name="cvt4")
            nc.vector.tensor_copy(cvt4[:], zwb0)
            dump("zwb0", cvt4[:])
            dump("cv0a", cv0a.bitcast(F32))
            dump("g00", gps[0][0][:].bitcast(F32))
            dump("r00", rs[0][0][:].bitcast(F32))
            dump("bv0", bvs[0][:])
            dump("g2", g2all[:].bitcast(F32))
            dump("g10", gps[1][0][:].bitcast(F32))
            dump("r10", rs[1][0][:].bitcast(F32))
            dump("r12", rs[1][2][:].bitcast(F32))
            dump("bv2d", bv2[:])
            dump("rs2", rs2[:].bitcast(F32))
            dump("scat", s_cat[:].bitcast(F32))
            dump("esb", e_sb[:].rearrange("p a b -> p (a b)"))
            dump("prt", prt_sb[:].rearrange("p a b c -> p (a b c)"))
            dump("idxs", idxs[:])

    nc.finalize()
    _prune_redundant_act_loads(nc)
    return nc


def _host_weights(Wd, bd, Ws, bs, Wg, bg, Wc, bc, Wa, ba, v, emb_perm):
    """Build wh/wm/wm2/wm3 (plain f32 arrays for the f32r tensors) and the
    eg gather rows (bf16) for one batch element."""
    f32 = np.float32
    bf = ml_dtypes.bfloat16
    dinv = f32(1.0) / np.sqrt(f32(513.0))
    c2 = f32(dinv * dinv)

    Mcs, brs = [], []
    for l in range(L):
        M = (Ws[l] @ Wg[:FD] + Wd[l] @ Wg[FD:]).astype(f32)
        Mcs.append((c2 * M).astype(f32))
        brs.append((bs[l] @ Wg[:FD] + bd[l] @ Wg[FD:] + bg).astype(f32))

    def blockdiag(M, nblk, rows=128):
        out = np.zeros((rows, 32 * nblk), f32)
        for q in range(nblk):
            out[32 * q:32 * (q + 1), 32 * q:32 * (q + 1)] = M
        return out

    def conv_merged(l):
        A = np.zeros((128, 128), f32)
        B = np.zeros((128, 128), f32)
        for (q_in, q_out, k, carry) in _conv_cells(l):
            blk = Wc[l][:, :, 0, k].T.astype(f32)
            dst = A if carry == 0 else B
            assert carry in (0, -1)
            assert not dst[32 * q_in:32 * (q_in + 1),
                           32 * q_out:32 * (q_out + 1)].any()
            dst[32 * q_in:32 * (q_in + 1), 32 * q_out:32 * (q_out + 1)] = blk
        return A, B

    embc = []
    for c in range(NTHI):
        tile_c = np.zeros((128, N), f32)
        for p in range(128):
            t = 4 * c + (3 - p // 32)
            tile_c[p] = emb_perm[t, p % 32]
        embc.append(tile_c)
    # colsum of the bf16-rounded tiles (what the device matmuls actually see)
    ecs_host = [e.astype(bf).astype(f32).sum(axis=1) for e in embc]

    wh = np.zeros((128, WH_COLS), f32)
    wh[:, ZR0:ZR0 + 128] = blockdiag(Mcs[0], 4)
    for l in range(L):
        wh[l, BRT:BRT + 128] = np.tile(brs[l], 4)
    for c in range(NTHI):
        wh[:, ECS + 2 * c] = ecs_host[c]
    wh[0:32, IDN:IDN + 32] = np.eye(32, dtype=f32)
    wfc_host = np.zeros((128, 4), f32)
    for l in range(L):
        wfc_host[:, l] = np.tile(bc[l].astype(f32), 4)
    wfc_host[0:32, 3] = ba.astype(f32)
    wh[:, WFC:WFC + 4] = wfc_host
    for l in range(L):
        wh[l, OH + l] = 1.0
    wh[0:32, VV] = v[:, 0].astype(f32)
    for c in range(NTHI):
        wh[32 * c:32 * (c + 1), ECS2] = ecs_host[c][0:32]

    wm = np.zeros((128, WM_COLS), f32)
    A0, B0 = conv_merged(0)
    wm[:, 0:128] = A0
    wm[:, 128:256] = B0
    wm[:, 256:384] = blockdiag(Mcs[1], 4)

    wm2 = np.zeros((128, WM2_COLS), f32)
    A1, B1 = conv_merged(1)
    wm2[:, 0:128] = A1
    wm2[:, 128:256] = B1
    wm2[0:96, 256:352] = blockdiag(Mcs[2], 3, rows=96)
    wm2[0:32, 352:384] = Wa.astype(f32)

    wm3 = np.zeros((128, WM3_COLS), f32)
    for c in range(NTHI):
        wm3[0:32, 96 * c + 32 * c:96 * c + 32 * (c + 1)] = Mcs[2]
    for k in range(K):
        wm3[32 * k:32 * (k + 1), 288:320] = Wc[2][:, :, 0, k].T
    wm3[0, 320:416] = np.tile(brs[2], 3)

    eg = np.zeros((352, 512), bf)
    eg[0:128] = embc[1].astype(bf)
    eg[128:256] = embc[2].astype(bf)
    for c in range(NTHI):
        eg[256 + 32 * c:256 + 32 * (c + 1)] = embc[c][0:32].astype(bf)
    e0 = embc[0].astype(bf)
    return wh, eg, wm, wm2, wm3, e0


def kernel(**inputs):
    node_embeddings = np.asarray(inputs["node_embeddings"], dtype=np.float32)
    args = tuple(np.asarray(inputs[k], np.float32) for k in
                 ("Wd", "bd", "Ws", "bs", "Wg", "bg", "Wc", "bc", "Wa", "ba", "v"))

    if "nc" not in _CACHE:
        _CACHE["nc"] = _build_nc()
    nc = _CACHE["nc"]

    n_cores = 8
    in_maps = []
    for i in range(n_cores):
        wh, eg, wm, wm2, wm3, e0 = _host_weights(*args,
                                                 node_embeddings[i % BSZ])
        in_maps.append({"wh": wh, "eg": eg, "wm": wm, "wm2": wm2, "wm3": wm3,
                        "e0": e0})
    res = run_bass_kernel_spmd(nc, in_maps, core_ids=list(range(n_cores)))
    y = np.stack([res.results[b]["y"] for b in range(BSZ)], axis=0)
    return y.astype(np.float32)
